# revision 4
# baseline (speedup 1.0000x reference)
"""CorrelationDimensionLoss kernel for 8x Trainium2 NeuronCores (Bass, raw engine programming).

Math:
  reference computes S_m = sum_{i<j} sigmoid(K*(r_m - d_ij)) / cnt for 16 log-spaced
  thresholds r_m, then -slope of lstsq(log r, log S).

Device strategy (identical SPMD program on 8 cores, different data):
  - The 8192x8192 pairwise-distance matrix is covered by its 8x8 grid of 1024x1024
    blocks; only the upper triangle incl. diagonal (36 blocks = 72 chunks of
    1024x512) is computed. Each core gets 9 chunks: its 2 diagonal-block chunks
    (counted x0.5 on host, self-pairs masked out) + 7 off-diagonal chunks (x1).
  - Per chunk: PE computes d^2 via one K=34 augmented matmul per 128-row tile
    ([-2x_i, |x_i|^2, 1] . [x_j, 1, |x_j|^2]), DVE clamps to >=0 into SBUF
    (adds +1e9 on self-pair diagonals), ACT takes sqrt -> d.
  - ACT then runs, per super-iteration of 2 chunks ([128, 8192] fp32 d tile):
      2 exp passes  e^{-10(d-2)}, e^{-20(d-2)}  -> tail thresholds (r <= d_min-0.45)
        via the expansion sigmoid(z) = e^z - e^{2z} + O(e^{3z})
      one sigmoid pass per middle threshold  sigmoid(-10 d + 10 r_m)
    each with fused accum_out (free-dim sum per partition) into an accumulator
    column. Thresholds with 10*(r_m - d_max) >= 18 saturate to exactly 1.0 in
    fp32 (as in the reference), so S_m = cnt with no device pass.
  - Host gathers the [128, n_pass*5] accumulators from all 8 cores, reduces in
    fp64, assembles the 16 sums and does the tiny lstsq.
"""

import os
import numpy as np

import concourse.bass as bass
import concourse.mybir as mybir
from concourse.bass_utils import run_bass_kernel_spmd

N = 8192
D = 32
NC = 8
KSHARP = 10.0
BLK = 1024          # row-block size (N / NC)
CHW = 512           # chunk width (cols)
NCHUNK = 9          # chunks per core: 2 diag + 7 offdiag
SUP_CHUNKS = [2, 2, 2, 2, 1]   # chunks per super-iteration
EXP_SHIFT = 2.0     # e^{-10(d-EXP_SHIFT)} scaling to keep T healthy in fp32
MASK_BIG = 1e9
TAIL_MARGIN = 0.45  # r_m <= d_min - TAIL_MARGIN -> 2-term exp expansion (rel err <= e^{-20*margin} ~ 1.2e-4)
SAT_Z = 18.0        # 10*(r_m - d_max) >= SAT_Z -> sigmoid == 1.0f exactly

_cache = {}

# exported for test.py
last_results = None
last_in_maps = None


def _chunk_assignment():
    """72 chunks -> 8 cores x 9 chunks. Chunk = (rowblock, col_offset_in_512units).
    Per core: chunks 0,1 are its diagonal-block halves; 2..8 are off-diagonal."""
    offdiag = []
    for i in range(NC):
        for j in range(i + 1, NC):
            for h in range(2):
                offdiag.append((i, 2 * j + h))
    assert len(offdiag) == 56
    assign = []
    for c in range(NC):
        mine = [(c, 2 * c), (c, 2 * c + 1)] + offdiag[7 * c:7 * c + 7]
        assign.append(mine)
    return assign


def _build_program(n_mid):
    """Identical-per-core Bass program. n_mid = number of direct-sigmoid thresholds."""
    n_pass = 2 + n_mid
    outc = n_pass * len(SUP_CHUNKS)
    nbias = 2 + n_mid
    f32 = mybir.dt.float32
    AF = mybir.ActivationFunctionType

    nc = bass.Bass("TRN2", target_bir_lowering=False, debug=False)
    rows_d = nc.dram_tensor("rows", [D + 2, NCHUNK * BLK], f32, kind="ExternalInput").ap()
    cols_d = nc.dram_tensor("cols", [D + 2, NCHUNK * CHW], f32, kind="ExternalInput").ap()
    mask_d = nc.dram_tensor("mask", [128, 128], f32, kind="ExternalInput").ap()
    bias_d = nc.dram_tensor("bias", [128, nbias], f32, kind="ExternalInput").ap()
    out_d = nc.dram_tensor("out", [128, outc], f32, kind="ExternalOutput").ap()

    W2 = 2 * 8 * CHW  # 8192: free width of a 2-chunk super-iteration

    from contextlib import ExitStack
    with ExitStack() as ctx:
        rows = ctx.enter_context(nc.sbuf_tensor("rows_sb", [D + 2, NCHUNK * BLK], f32)).ap()
        cols = ctx.enter_context(nc.sbuf_tensor("cols_sb", [D + 2, NCHUNK * CHW], f32)).ap()
        mask = ctx.enter_context(nc.sbuf_tensor("mask_sb", [128, 128], f32)).ap()
        bias = ctx.enter_context(nc.sbuf_tensor("bias_sb", [128, nbias], f32)).ap()
        d2 = ctx.enter_context(nc.sbuf_tensor("d2_sb", [128, W2], f32)).ap()
        dd = ctx.enter_context(nc.sbuf_tensor("d_sb", [128, W2], f32)).ap()
        scr = ctx.enter_context(nc.sbuf_tensor("scr_sb", [128, W2], mybir.dt.bfloat16)).ap()
        acc = ctx.enter_context(nc.sbuf_tensor("acc_sb", [128, outc], f32)).ap()
        psum = [ctx.enter_context(nc.psum_tensor(f"ps{i}", [128, CHW], f32)).ap() for i in range(8)]
        dma_sem = ctx.enter_context(nc.semaphore("dma_sem"))
        pe_sem = ctx.enter_context(nc.semaphore("pe_sem"))
        dve_sem = ctx.enter_context(nc.semaphore("dve_sem"))
        sqrt_sem = ctx.enter_context(nc.semaphore("sqrt_sem"))
        done_sem = ctx.enter_context(nc.semaphore("done_sem"))
        block = ctx.enter_context(nc.Block())

        @block.gpsimd
        def _(g):
            g.dma_start(out=rows, in_=rows_d).then_inc(dma_sem, 16)
            g.dma_start(out=cols, in_=cols_d).then_inc(dma_sem, 16)
            g.dma_start(out=mask, in_=mask_d).then_inc(dma_sem, 16)
            g.dma_start(out=bias, in_=bias_d).then_inc(dma_sem, 16)
            g.wait_ge(done_sem, 1)
            g.dma_start(out=out_d, in_=acc).then_inc(dma_sem, 16)

        @block.tensor
        def _(t):
            t.wait_ge(dma_sem, 64)
            k = 0
            for s, nch in enumerate(SUP_CHUNKS):
                for _ in range(nch):
                    if k > 0:
                        t.wait_ge(dve_sem, k)  # prior chunk drained from PSUM
                    mm = None
                    for ti in range(8):
                        mm = t.matmul(
                            psum[ti],
                            lhsT=rows[:, BLK * k + 128 * ti: BLK * k + 128 * (ti + 1)],
                            rhs=cols[:, CHW * k: CHW * (k + 1)],
                            start=True, stop=True,
                        )
                    mm.then_inc(pe_sem, 1)
                    k += 1

        @block.vector
        def _(v):
            k = 0
            for s, nch in enumerate(SUP_CHUNKS):
                for ci in range(nch):
                    v.wait_ge(pe_sem, k + 1)
                    if ci == 0 and s >= 1:
                        v.wait_ge(sqrt_sem, s)  # d2 buffer free (sqrt of prev super-iter done)
                    base = 4096 * ci
                    for ti in range(8):
                        op = v.tensor_scalar_max(d2[:, base + CHW * ti: base + CHW * (ti + 1)], psum[ti], 0.0)
                    if s == 0:
                        # diagonal-block self-pair masking: chunk 0 -> tiles 0..3 at
                        # in-tile col 128*ti; chunk 1 -> tiles 4..7 at 128*(ti-4)
                        tis = range(0, 4) if ci == 0 else range(4, 8)
                        for ti in tis:
                            off = base + CHW * ti + 128 * (ti if ci == 0 else ti - 4)
                            op = v.tensor_add(d2[:, off:off + 128], d2[:, off:off + 128], mask)
                    op.then_inc(dve_sem, 1)
                    k += 1

        @block.scalar
        def _(sc):
            cum = 0
            for s, nch in enumerate(SUP_CHUNKS):
                cum += nch
                W = 4096 * nch
                sc.wait_ge(dve_sem, cum)
                sc.activation(dd[:, :W], d2[:, :W], AF.Sqrt).then_inc(sqrt_sem, 1)
                col = s * n_pass
                sc.activation(scr[:, :W], dd[:, :W], AF.Exp, scale=-KSHARP,
                              bias=bias[:, 0:1], accum_out=acc[:, col:col + 1])
                last = sc.activation(scr[:, :W], dd[:, :W], AF.Exp, scale=-2.0 * KSHARP,
                                     bias=bias[:, 1:2], accum_out=acc[:, col + 1:col + 2])
                for i in range(n_mid):
                    last = sc.activation(scr[:, :W], dd[:, :W], AF.Sigmoid, scale=-KSHARP,
                                         bias=bias[:, 2 + i:3 + i],
                                         accum_out=acc[:, col + 2 + i:col + 3 + i])
                if s == len(SUP_CHUNKS) - 1:
                    last.then_inc(done_sem, 1)
    return nc


def _dist_extremes(pts):
    """min (off-diagonal) and max pairwise distance, blocked fp32 numpy."""
    sq = np.einsum("ij,ij->i", pts, pts)
    dmin = np.inf
    dmax = 0.0
    B = 1024
    for i0 in range(0, N, B):
        g = pts[i0:i0 + B] @ pts.T
        d2b = sq[i0:i0 + B, None] + sq[None, :] - 2.0 * g
        for r in range(d2b.shape[0]):
            d2b[r, i0 + r] = np.inf
        dmin = min(dmin, float(np.sqrt(max(d2b.min(), 0.0))))
        for r in range(d2b.shape[0]):
            d2b[r, i0 + r] = 0.0
        dmax = max(dmax, float(np.sqrt(max(d2b.max(), 0.0))))
    return dmin, dmax


def kernel(points, r_values):
    global last_results
    points = np.ascontiguousarray(np.asarray(points, dtype=np.float32))
    r_values = np.asarray(r_values, dtype=np.float32)
    assert points.shape == (N, D) and r_values.shape == (16,)
    rv = r_values.astype(np.float64)

    dmin, dmax = _dist_extremes(points)

    tail = [m for m in range(16) if rv[m] <= dmin - TAIL_MARGIN]
    sat = [m for m in range(16) if KSHARP * (rv[m] - dmax) >= SAT_Z]
    mid = [m for m in range(16) if m not in tail and m not in sat]
    n_mid = len(mid)

    key = n_mid
    if key not in _cache:
        _cache[key] = _build_program(n_mid)
    nc = _cache[key]
    n_pass = 2 + n_mid

    # host-side augmented matrices:  d2 = a_i . b_j
    sq = np.einsum("ij,ij->i", points, points).astype(np.float32)
    ones = np.ones(N, dtype=np.float32)
    A = np.concatenate([(-2.0 * points).T, sq[None, :], ones[None, :]], axis=0)  # [34, N] lhsT source
    B = np.concatenate([points.T, ones[None, :], sq[None, :]], axis=0)          # [34, N] rhs source

    assign = _chunk_assignment()
    in_maps = []
    maskarr = (MASK_BIG * np.eye(128, dtype=np.float32))
    biasarr = np.zeros((128, 2 + n_mid), dtype=np.float32)
    biasarr[:, 0] = KSHARP * EXP_SHIFT
    biasarr[:, 1] = 2.0 * KSHARP * EXP_SHIFT
    for i, m in enumerate(mid):
        biasarr[:, 2 + i] = KSHARP * r_values[m]
    for c in range(NC):
        rows = np.empty((D + 2, NCHUNK * BLK), dtype=np.float32)
        colsb = np.empty((D + 2, NCHUNK * CHW), dtype=np.float32)
        for k, (rb, ch) in enumerate(assign[c]):
            rows[:, k * BLK:(k + 1) * BLK] = A[:, rb * BLK:(rb + 1) * BLK]
            colsb[:, k * CHW:(k + 1) * CHW] = B[:, ch * CHW:(ch + 1) * CHW]
        in_maps.append({"rows": rows, "cols": colsb, "mask": maskarr, "bias": biasarr})

    global last_in_maps
    last_in_maps = in_maps
    trace = bool(os.environ.get("CDL_TRACE"))
    res = run_bass_kernel_spmd(nc, in_maps, core_ids=list(range(NC)), trace=trace)
    last_results = res

    # fp64 host combine
    totals = np.zeros(n_pass, dtype=np.float64)
    for c in range(NC):
        accm = res.results[c]["out"].astype(np.float64)  # [128, n_pass*5]
        for s in range(len(SUP_CHUNKS)):
            w = 0.5 if s == 0 else 1.0
            totals += w * accm[:, s * n_pass:(s + 1) * n_pass].sum(axis=0)

    cnt = N * (N - 1) / 2.0
    S = np.zeros(16, dtype=np.float64)
    T1, T2 = totals[0], totals[1]
    for m in tail:
        S[m] = (np.exp(KSHARP * (rv[m] - EXP_SHIFT)) * T1
                - np.exp(2.0 * KSHARP * (rv[m] - EXP_SHIFT)) * T2)
    for i, m in enumerate(mid):
        S[m] = totals[2 + i]
    for m in sat:
        S[m] = cnt

    corr = S / cnt
    logr = np.log(rv)
    logc = np.log(corr)
    Amat = np.stack([logr, np.ones_like(logr)], axis=1)
    sol = np.linalg.solve(Amat.T @ Amat, Amat.T @ logc)
    return np.asarray(-sol[0], dtype=np.float32)
